# revision 4
# baseline (speedup 1.0000x reference)
"""Cost-volume concat kernel for Trainium2 (8 NeuronCores, SPMD over H).

Problem: un_l, un_r [1, 16, 128, 512] f32 ->
         out [1, 32, 96, 128, 512] f32 where
  out[:, :16, d]  = un_l                       (broadcast over d)
  out[:, 16:, d]  = roll(un_r, +d, axis=W)     (width roll per disparity)

Pure data movement (~805 MB of output writes); memory-bound. Each core
handles 16 rows of H (H=128 sharded over 8 cores) and writes ~100.7 MB.
Everything inside a core is DMA:

  - SBUF layout: partition = h*8 + c_local. Each per-channel store DMA
    then spans partitions {q, 8+q, 16+q, ...}, which map to 8 of the 16
    SDMA engines; channels with q<4 and q>=4 cover complementary engine
    halves. Inputs are pre-permuted on the host into this partition
    order so loads are plain contiguous 128-partition DMAs.
  - r tiles are doubled along W ([128, 1024]): the rolled row for
    disparity d is the contiguous window [512-d, 1024-d); a source AP
    step of -1 over d folds all 96 disparities into one DMA per channel.
  - l-part uses a zero-step AP dim over d (broadcast to all 96 slots).
  - Stores are issued on BOTH HWDGE rings: q<4 channels on the sync
    ring, q>=4 on the scalar ring, so the two engine halves stream
    concurrently (~2x over a single ring).

32 store DMAs x ~12 MB per core; measured ~270-310 us/core store phase
(~370 GB/s/core), at the per-core HBM write roofline.
"""
import sys

if "/opt/trn_rl_repo" not in sys.path:
    sys.path.insert(0, "/opt/trn_rl_repo")

import numpy as np
import concourse.bass as bass
from concourse import mybir
from concourse.bass_utils import run_bass_kernel_spmd

B, C, H, W, D = 1, 16, 128, 512, 96
N_CORES = 8
HL = H // N_CORES  # 16 rows per core


def _build():
    Hl = HL
    nc = bass.Bass()
    # host-permuted inputs: x[t, p, w] with p = h*8 + q, channel = 8t + q
    l = nc.dram_tensor("l", [2, 128, W], mybir.dt.float32, kind="ExternalInput")
    r = nc.dram_tensor("r", [2, 128, W], mybir.dt.float32, kind="ExternalInput")
    out = nc.dram_tensor(
        "out", [2 * C, D, Hl, W], mybir.dt.float32, kind="ExternalOutput"
    )

    s_c = D * Hl * W  # out strides (elements)
    s_d = Hl * W
    s_h = W

    with (
        nc.sbuf_tensor("l0", [128, W], mybir.dt.float32) as l0,
        nc.sbuf_tensor("l1", [128, W], mybir.dt.float32) as l1,
        nc.sbuf_tensor("r0", [128, 2 * W], mybir.dt.float32) as r0,
        nc.sbuf_tensor("r1", [128, 2 * W], mybir.dt.float32) as r1,
        nc.semaphore("l_sem") as l_sem,
        nc.semaphore("r_sem") as r_sem,
        nc.semaphore("store_sem") as store_sem,
        nc.semaphore("store_sem2") as store_sem2,
        nc.Block() as block,
    ):
        def emit_l_store(eng, c, sem):
            t, q = c // 8, c % 8
            lt = (l0, l1)[t]
            eng.dma_start(
                bass.AP(out, c * s_c, [[s_h, Hl], [s_d, D], [1, W]]),
                bass.AP(lt, q * W, [[8 * W, Hl], [0, D], [1, W]]),
            ).then_inc(sem, 16)

        def emit_r_store(eng, c, sem):
            t, q = c // 8, c % 8
            rt = (r0, r1)[t]
            eng.dma_start(
                bass.AP(out, (C + c) * s_c, [[s_h, Hl], [s_d, D], [1, W]]),
                bass.AP(rt, q * 2 * W + W, [[16 * W, Hl], [-1, D], [1, W]]),
            ).then_inc(sem, 16)

        @block.sync
        def _(sync):
            for t, lt in enumerate((l0, l1)):
                sync.dma_start(
                    bass.AP(lt, 0, [[W, 128], [1, W]]), l[t]
                ).then_inc(l_sem, 16)
            for t, rt in enumerate((r0, r1)):
                for rep in range(2):
                    sync.dma_start(
                        bass.AP(rt, rep * W, [[2 * W, 128], [1, W]]), r[t]
                    ).then_inc(r_sem, 16)
            # engine half A: q in 0..3, both tiles; l-stores first (only
            # need l tiles), r-stores after r tiles land
            chans = [t * 8 + j for j in range(4) for t in range(2)]
            n = 0
            sync.wait_ge(l_sem, 32)
            for c in chans:
                emit_l_store(sync, c, store_sem)
                n += 1
            sync.wait_ge(r_sem, 64)
            for c in chans:
                emit_r_store(sync, c, store_sem)
                n += 1
            sync.wait_ge(store_sem, 16 * n)

        @block.scalar
        def _(scalar):
            # engine half B: q in 4..7
            chans = [t * 8 + 4 + j for j in range(4) for t in range(2)]
            n = 0
            scalar.wait_ge(l_sem, 32)
            for c in chans:
                emit_l_store(scalar, c, store_sem2)
                n += 1
            scalar.wait_ge(r_sem, 64)
            for c in chans:
                emit_r_store(scalar, c, store_sem2)
                n += 1
            scalar.wait_ge(store_sem2, 16 * n)

    return nc


_nc = None


def _get_nc():
    global _nc
    if _nc is None:
        _nc = _build()
    return _nc


def _permute(shard):
    # shard [C, HL, W] -> [2, 128, W] with row p = h*8 + q, channel = 8t + q
    x = shard.reshape(2, 8, HL, W)          # [t, q, h, w]
    x = x.transpose(0, 2, 1, 3)             # [t, h, q, w]
    return np.ascontiguousarray(x.reshape(2, 128, W))


def kernel(un_l, un_r, **run_kwargs):
    un_l = np.ascontiguousarray(np.asarray(un_l), dtype=np.float32)
    un_r = np.ascontiguousarray(np.asarray(un_r), dtype=np.float32)
    assert un_l.shape == (B, C, H, W) and un_r.shape == (B, C, H, W)

    in_maps = [
        {
            "l": _permute(un_l[0, :, k * HL : (k + 1) * HL, :]),
            "r": _permute(un_r[0, :, k * HL : (k + 1) * HL, :]),
        }
        for k in range(N_CORES)
    ]
    res = run_bass_kernel_spmd(
        _get_nc(), in_maps, core_ids=list(range(N_CORES)), **run_kwargs
    )
    out = np.empty((B, 2 * C, D, H, W), np.float32)
    for k in range(N_CORES):
        out[0, :, :, k * HL : (k + 1) * HL, :] = res.results[k]["out"]
    if run_kwargs:
        return out, res
    return out
